# revision 22
# baseline (speedup 1.0000x reference)
"""NVFP4 quantized linear (simulated) on 8 TRN2 NeuronCores.

out = dq(quant_nvfp4(x)) @ dq(quant_nvfp4(w)).T

Sharding: K-parallel (contraction dim). Core c gets x[:, 512c:512c+512]
and weight[:, 512c:512c+512]. NVFP4 blocks are 32 wide along K, so
quantization is fully local to a K-slice for BOTH operands; there are no
collectives at all (no AllGather barrier, no launch-skew sensitivity).
Each core computes the full-size partial product over its K-slice,
drains it transposed ([N, M] fp16), and the host sums the 8 partials
(the unshard step) and transposes back.

Quant runs on [128, 8, 512] megachunks (one DMA load each; 1 for x, 4
for w). Palette rounding (NVFP4_Q custom DVE op, 8 stages): v = x*r6;
hi = Veltkamp 2-sig-bit round; t = select(v^2<=4, v, hi). The remaining
snap-to-0.5-grid + dequant scale is routed per-megachunk to balance
engines:
  - DVE route: one 3-stage custom op
        dq16 = ((t + 1.5*2^22) - 1.5*2^22) * scl_bcast
    (the fp32 magic add rounds t to the 0.5 grid; DVE stages round to
    fp32 like the Veltkamp stages do).
  - Scalar+Pool route: y16 = fp16(t + 768) — 1.5*2^9 makes the fp16
    downconvert round to the 0.5 grid — then z16 = y16 - 768 (exact) on
    Scalar, and dq = z16 * scl_bcast on GpSimd.

The transposed operands use a tile-blocked layout (tile (f, s) of a
megachunk lands at flat offset 128*(4f+s)) so one xbar-transpose DMA
moves a whole megachunk — 5 transpose triggers total instead of 40
(HWDGE trigger cost ~1.2us each on the sync queue was serializing the
pipeline). Matmuls read the blocked layout via strided APs; PSUM holds
two bands per tile (4 banks) so drains/stores batch pairwise.
"""

import dataclasses
import sys

import numpy as np

if "/opt/trn_rl_repo" not in sys.path:
    sys.path.insert(0, "/opt/trn_rl_repo")

from concourse import bacc, mybir
from concourse import dve_ops as _dve_ops
import concourse.bass as bass  # noqa: F401
import concourse.tile as tile
import concourse.bass_utils as bass_utils
from concourse.dve_spec import Spec, Src0, Src1, C0, C1, select, sq, lower
from concourse.dve_uop import DveOpSpec

M, K, N = 1024, 4096, 4096
NCORES = 8
KLOC = K // NCORES  # 512 contraction elements per core
BS = 32
NSL = KLOC // 128  # 4 transposed k-slices per core
MEGA = 8  # row-tiles per megachunk
NBM = MEGA * KLOC // BS  # 128 blocks per megachunk
WCH = N // (128 * MEGA)  # 4 w megachunks

FP32 = mybir.dt.float32
FP16 = mybir.dt.float16
Alu = mybir.AluOpType
AX = mybir.AxisListType

C_VELT = 4194305.0  # 2^22 + 1: Veltkamp split -> 2 significant bits
C_FIX32 = 6291456.0  # 1.5 * 2^22: fp32 magic add rounds to the 0.5-grid
C_FIX16 = 768.0  # 1.5 * 2^9: fp16 magic; downconvert snaps to 0.5-grid

_NC_CACHE = {}


def _nvfp4_ref(in0, in1, c0, c1, c2):
    f32 = np.float32
    x = np.asarray(in0, np.float32)
    r6 = np.asarray(in1, np.float32)
    if r6.shape != x.shape:
        if r6.ndim == 3:
            r6 = r6[..., 0]
        bs = x.size // r6.size
        r6 = np.repeat(r6, bs, axis=-1).reshape(x.shape)
    v = (x * r6).astype(np.float32)
    c = (v * f32(c1)).astype(np.float32)
    d = (c - v).astype(np.float32)
    hi = (c - d).astype(np.float32)
    return np.where(v * v <= np.asarray(c0, np.float32), v, hi).astype(np.float32)


def _fixdq_ref(in0, in1, c0, c1, c2):
    t = np.asarray(in0, np.float32)
    s = np.asarray(in1, np.float32)
    if s.shape != t.shape:
        if s.ndim == 3:
            s = s[..., 0]
        bs = t.size // s.size
        s = np.repeat(s, bs, axis=-1).reshape(t.shape)
    c = np.float32(c0)
    a = (t + c).astype(np.float32)
    b = (a - c).astype(np.float32)
    return (b * s).astype(np.float32)


def _register_op(name, spec):
    if name in _dve_ops._SUB_OPCODE_FOR_NAME:
        return next(o for o in _dve_ops.OPS if o.name == name)
    op = _dve_ops.DveOp(name, spec, subdim=False, uops_sha={})
    _dve_ops.OPS.append(op)
    _dve_ops.CUSTOM_DVE_SPECS[name] = spec
    row = _dve_ops._CUSTOM_DVE_ROW_BASE + len(_dve_ops.OPS) - 1
    _dve_ops._SUB_OPCODE_FOR_NAME[name] = row
    shas = {}
    for ver in ("v3",):
        s = DveOpSpec(name=name, opcode=row, uops=lower(spec, ver=ver), rd1_en=True)
        shas[ver] = s.sha(ver)
    op = dataclasses.replace(op, uops_sha=shas)
    _dve_ops.OPS[-1] = op
    _dve_ops.CUSTOM_DVE_SPECS[name] = op.spec
    return op


def _make_ops():
    _v = Src0 * Src1
    _c = _v * C1
    _d = _c - _v
    _hi = _c - _d
    _m = sq(_v) <= C0
    q = _register_op("NVFP4_Q_ANT", Spec(body=select(_m, _v, _hi), reference=_nvfp4_ref))
    _a = Src0 + C0
    _b = _a - C0
    f = _register_op("NVFP4_FIXDQ_ANT", Spec(body=_b * Src1, reference=_fixdq_ref))
    return q, f


NVFP4_Q, NVFP4_FIXDQ = _make_ops()


def _quant_chunk(nc, pools, xt, dst_t, base, nf):
    """Quantize a pre-loaded [128, nf, KLOC] fp32 tile and transpose it
    into the tile-blocked dst_t[:, base : base + nf, :, :] in ONE xbar
    DMA (fused fix + dequant on the DVE)."""
    io, workt, work, small = pools
    nb = nf * KLOC // BS
    x4 = xt.rearrange("p f (nb b) -> p f nb b", b=BS)
    x3 = xt.rearrange("p f (nb b) -> p (f nb) b", b=BS)

    bmax = small.tile([128, nb], FP32, name=f"bmax{nf}", tag=f"bmax{nf}")
    nc.vector.tensor_reduce(
        bmax.rearrange("p (f nb) -> p f nb", f=nf),
        x4,
        axis=AX.X,
        op=Alu.max,
        apply_absolute_value=True,
    )
    scl = small.tile([128, nb], FP32, name=f"scl{nf}", tag=f"scl{nf}")
    nc.vector.tensor_scalar(scl, bmax, 1e-12, 1.0 / 6.0, Alu.max, Alu.mult)
    r6 = small.tile([128, nb], FP32, name=f"r6{nf}", tag=f"r6{nf}")
    nc.vector.reciprocal_approx_fast(r6, scl)

    # bufs=1: the WAR hazard on the single t buffer pins the scheduler to
    # strict dve1 -> dve2 pairing per chunk (it otherwise interleaves
    # chunks on the DVE queue, delaying the chunk's transpose)
    t = workt.tile([128, nf, KLOC], FP32, name=f"t{nf}", tag=f"t{nf}")
    t3 = t.rearrange("p f (nb b) -> p (f nb) b", b=BS)
    nc.vector._custom_dve(
        NVFP4_Q,
        out=t3,
        in0=x3,
        in1=r6.unsqueeze(2).broadcast_to((128, nb, BS)),
        s0=4.0,
        s1=C_VELT,
    )

    dq = work.tile([128, nf, KLOC], FP16, name=f"dq{nf}", tag=f"dq{nf}")
    dq3 = dq.rearrange("p f (nb b) -> p (f nb) b", b=BS)
    scl_b = scl.unsqueeze(2).broadcast_to((128, nb, BS))
    nc.vector._custom_dve(NVFP4_FIXDQ, out=dq3, in0=t3, in1=scl_b, s0=C_FIX32)

    # one whole-chunk xbar transpose: source tile i = in[:, 128i:128i+128]
    # lands at dst tile i, i.e. tile (f, s) at [:, base + f, s, :].
    nc.sync.dma_start_transpose(
        dst_t[:, base : base + nf, :, :].rearrange("p f s c -> p (f s) c"),
        dq.rearrange("p f k -> p (f k)"),
    )


def _body(nc, tc, x_d, w_d, o_d):
    with (
        tc.tile_pool(name="persist", bufs=1) as persist,
        tc.tile_pool(name="io", bufs=3) as io,
        tc.tile_pool(name="workt", bufs=1) as workt,
        tc.tile_pool(name="work", bufs=2) as work,
        tc.tile_pool(name="small", bufs=2) as small,
        tc.tile_pool(name="out", bufs=3) as outp,
        tc.tile_pool(name="psum", bufs=1, space="PSUM") as psum_pool,
    ):
        # tile-blocked transposed operands: tile (row-tile f, k-slice s)
        # lives at [:, f, s, :] — flat offset 512f + 128s, matching the
        # source-tile order of a whole-mega xbar transpose
        xdqT = persist.tile([128, MEGA, NSL, 128], FP16)
        wdqT = persist.tile([128, N // 128, NSL, 128], FP16)
        pools = (io, workt, work, small)

        # loads on the SWDGE queue: x halves first (they gate the first
        # matmuls), then w megas; the SWDGE descriptor rings drain in
        # issue order so earlier loads complete first
        def _load(eng, src, nf, name):
            r = io.tile([128, nf, KLOC], FP32, name=name, tag=f"raw{nf}")
            eng.dma_start(r, src.rearrange("(f p) k -> p f k", p=128))
            return r

        # first loads via HWDGE (fast start on the idle scalar queue);
        # the rest via SWDGE where they drain in issue order
        xh0 = _load(nc.scalar, x_d[0:512, :], 4, "xh0")
        w_raws = [None] * WCH
        w_raws[0] = _load(nc.scalar, w_d[0 : 128 * MEGA, :], MEGA, "wraw0")
        xh1 = _load(nc.scalar, x_d[512:1024, :], 4, "xh1")
        for j in range(1, WCH):
            w_raws[j] = _load(
                nc.gpsimd, w_d[128 * MEGA * j : 128 * MEGA * (j + 1), :], MEGA, f"wraw{j}"
            )

        _quant_chunk(nc, pools, xh0, xdqT, 0, 4)
        _quant_chunk(nc, pools, w_raws[0], wdqT, 0, MEGA)
        _quant_chunk(nc, pools, xh1, xdqT, 4, 4)

        for j in range(WCH):
            if j > 0:
                _quant_chunk(nc, pools, w_raws[j], wdqT, MEGA * j, MEGA)
            for tp in range(4 * j, 4 * (j + 1)):  # band pairs
                ps = psum_pool.tile(
                    [128, 2 * M], FP32, name=f"ps{tp % 2}", tag=f"ps{tp % 2}"
                )
                for ti in range(2):
                    t = 2 * tp + ti
                    for mh in range(2):
                        for s in range(NSL):
                            nc.tensor.matmul(
                                ps[:, ti * M + mh * 512 : ti * M + (mh + 1) * 512],
                                wdqT[:, t, s, :],
                                xdqT[:, 4 * mh : 4 * (mh + 1), s, :],
                                start=(s == 0),
                                stop=(s == NSL - 1),
                            )
                ot = outp.tile([128, 2 * M], FP16, name="ot", tag="ot")
                nc.scalar.copy(ot, ps)
                nc.gpsimd.dma_start(
                    o_d[256 * tp : 256 * (tp + 1), :].rearrange(
                        "(f p) m -> p f m", p=128
                    ),
                    ot.rearrange("p (f m) -> p f m", f=2),
                )


def _get_nc():
    if "nc" not in _NC_CACHE:
        nc = bacc.Bacc(
            "TRN2", target_bir_lowering=False, debug=False, num_devices=NCORES
        )
        x_d = nc.dram_tensor("x", (M, KLOC), FP32, kind="ExternalInput").ap()
        w_d = nc.dram_tensor("w", (N, KLOC), FP32, kind="ExternalInput").ap()
        o_d = nc.dram_tensor("out", (N, M), FP16, kind="ExternalOutput").ap()
        with tile.TileContext(nc) as tc:
            _body(nc, tc, x_d, w_d, o_d)
        nc.compile()
        _NC_CACHE["nc"] = nc
    return _NC_CACHE["nc"]


def kernel(x: np.ndarray, weight: np.ndarray, _trace: bool = False, **_):
    nc = _get_nc()
    x = np.ascontiguousarray(x, dtype=np.float32)
    weight = np.ascontiguousarray(weight, dtype=np.float32)
    in_maps = [
        {
            "x": x[:, c * KLOC : (c + 1) * KLOC],
            "w": weight[:, c * KLOC : (c + 1) * KLOC],
        }
        for c in range(NCORES)
    ]
    res = bass_utils.run_bass_kernel_spmd(
        nc, in_maps, list(range(NCORES)), trace=_trace
    )
    acc = np.zeros((N, M), dtype=np.float32)
    for c in range(NCORES):
        acc += res.results[c]["out"].astype(np.float32)
    if _trace:
        kernel.last_result = res
    return np.ascontiguousarray(acc.T, dtype=np.float32)


# revision 24
# speedup vs baseline: 1.0101x; 1.0101x over previous
"""NVFP4 quantized linear (simulated) on 8 TRN2 NeuronCores.

out = dq(quant_nvfp4(x)) @ dq(quant_nvfp4(w)).T

Sharding: K-parallel (contraction dim). Core c gets x[:, 512c:512c+512]
and weight[:, 512c:512c+512]. NVFP4 blocks are 32 wide along K, so
quantization is fully local to a K-slice for BOTH operands; there are no
collectives at all (no AllGather barrier, no launch-skew sensitivity).
Each core computes the full-size partial product over its K-slice,
drains it transposed ([N, M] fp16), and the host sums the 8 partials
(the unshard step) and transposes back.

Quant runs on [128, 8, 512] megachunks (one DMA load each; 1 for x, 4
for w). Palette rounding (NVFP4_Q custom DVE op, 8 stages): v = x*r6;
hi = Veltkamp 2-sig-bit round; t = select(v^2<=4, v, hi). The remaining
snap-to-0.5-grid + dequant scale is routed per-megachunk to balance
engines:
  - DVE route: one 3-stage custom op
        dq16 = ((t + 1.5*2^22) - 1.5*2^22) * scl_bcast
    (the fp32 magic add rounds t to the 0.5 grid; DVE stages round to
    fp32 like the Veltkamp stages do).
  - Scalar+Pool route: y16 = fp16(t + 768) — 1.5*2^9 makes the fp16
    downconvert round to the 0.5 grid — then z16 = y16 - 768 (exact) on
    Scalar, and dq = z16 * scl_bcast on GpSimd.

The transposed operands use a tile-blocked layout (tile (f, s) of a
megachunk lands at flat offset 128*(4f+s)) so one xbar-transpose DMA
moves a whole megachunk — 5 transpose triggers total instead of 40
(HWDGE trigger cost ~1.2us each on the sync queue was serializing the
pipeline). Matmuls read the blocked layout via strided APs; PSUM holds
two bands per tile (4 banks) so drains/stores batch pairwise.
"""

import dataclasses
import sys

import numpy as np

if "/opt/trn_rl_repo" not in sys.path:
    sys.path.insert(0, "/opt/trn_rl_repo")

from concourse import bacc, mybir
from concourse import dve_ops as _dve_ops
import concourse.bass as bass  # noqa: F401
import concourse.tile as tile
import concourse.bass_utils as bass_utils
from concourse.dve_spec import Spec, Src0, Src1, C0, C1, select, sq, lower
from concourse.dve_uop import DveOpSpec

M, K, N = 1024, 4096, 4096
NCORES = 8
KLOC = K // NCORES  # 512 contraction elements per core
BS = 32
NSL = KLOC // 128  # 4 transposed k-slices per core
MEGA = 8  # row-tiles per megachunk
NBM = MEGA * KLOC // BS  # 128 blocks per megachunk
WCH = N // (128 * MEGA)  # 4 w megachunks

FP32 = mybir.dt.float32
FP16 = mybir.dt.float16
Alu = mybir.AluOpType
AX = mybir.AxisListType

C_VELT = 4194305.0  # 2^22 + 1: Veltkamp split -> 2 significant bits
C_FIX32 = 6291456.0  # 1.5 * 2^22: fp32 magic add rounds to the 0.5-grid
C_FIX16 = 768.0  # 1.5 * 2^9: fp16 magic; downconvert snaps to 0.5-grid

_NC_CACHE = {}


def _nvfp4_ref(in0, in1, c0, c1, c2):
    f32 = np.float32
    x = np.asarray(in0, np.float32)
    r6 = np.asarray(in1, np.float32)
    if r6.shape != x.shape:
        if r6.ndim == 3:
            r6 = r6[..., 0]
        bs = x.size // r6.size
        r6 = np.repeat(r6, bs, axis=-1).reshape(x.shape)
    v = (x * r6).astype(np.float32)
    c = (v * f32(c1)).astype(np.float32)
    d = (c - v).astype(np.float32)
    hi = (c - d).astype(np.float32)
    return np.where(v * v <= np.asarray(c0, np.float32), v, hi).astype(np.float32)


def _fixdq_ref(in0, in1, c0, c1, c2):
    t = np.asarray(in0, np.float32)
    s = np.asarray(in1, np.float32)
    if s.shape != t.shape:
        if s.ndim == 3:
            s = s[..., 0]
        bs = t.size // s.size
        s = np.repeat(s, bs, axis=-1).reshape(t.shape)
    c = np.float32(c0)
    a = (t + c).astype(np.float32)
    b = (a - c).astype(np.float32)
    return (b * s).astype(np.float32)


def _register_op(name, spec):
    if name in _dve_ops._SUB_OPCODE_FOR_NAME:
        return next(o for o in _dve_ops.OPS if o.name == name)
    op = _dve_ops.DveOp(name, spec, subdim=False, uops_sha={})
    _dve_ops.OPS.append(op)
    _dve_ops.CUSTOM_DVE_SPECS[name] = spec
    row = _dve_ops._CUSTOM_DVE_ROW_BASE + len(_dve_ops.OPS) - 1
    _dve_ops._SUB_OPCODE_FOR_NAME[name] = row
    shas = {}
    for ver in ("v3",):
        s = DveOpSpec(name=name, opcode=row, uops=lower(spec, ver=ver), rd1_en=True)
        shas[ver] = s.sha(ver)
    op = dataclasses.replace(op, uops_sha=shas)
    _dve_ops.OPS[-1] = op
    _dve_ops.CUSTOM_DVE_SPECS[name] = op.spec
    return op


def _make_ops():
    _v = Src0 * Src1
    _c = _v * C1
    _d = _c - _v
    _hi = _c - _d
    _m = sq(_v) <= C0
    q = _register_op("NVFP4_Q_ANT", Spec(body=select(_m, _v, _hi), reference=_nvfp4_ref))
    _a = Src0 + C0
    _b = _a - C0
    f = _register_op("NVFP4_FIXDQ_ANT", Spec(body=_b * Src1, reference=_fixdq_ref))
    return q, f


NVFP4_Q, NVFP4_FIXDQ = _make_ops()


def _stats(nc, pools, xt, nf, uid):
    """Blockwise stats for a pre-loaded [128, nf, KLOC] fp32 tile:
    returns (scl, r6). Cheap; issued early so they hide in load gaps and
    keep the DVE-counter positions of the palette passes exact."""
    io, workt, work, small = pools
    nb = nf * KLOC // BS
    x4 = xt.rearrange("p f (nb b) -> p f nb b", b=BS)
    bmax = small.tile([128, nb], FP32, name=f"bmax{uid}", tag=f"bmax{nf}")
    nc.vector.tensor_reduce(
        bmax.rearrange("p (f nb) -> p f nb", f=nf),
        x4,
        axis=AX.X,
        op=Alu.max,
        apply_absolute_value=True,
    )
    scl = small.tile([128, nb], FP32, name=f"scl{uid}", tag=f"scl{uid}")
    nc.vector.tensor_scalar(scl, bmax, 1e-12, 1.0 / 6.0, Alu.max, Alu.mult)
    r6 = small.tile([128, nb], FP32, name=f"r6{uid}", tag=f"r6{nf}")
    nc.vector.reciprocal_approx_fast(r6, scl)
    return scl, r6


def _palette(nc, pools, xt, stats, dst_t, base, nf):
    """Palette-round + fix + dequant a [128, nf, KLOC] tile on the DVE and
    xbar-transpose it into the tile-blocked dst_t[:, base : base+nf]."""
    io, workt, work, small = pools
    nb = nf * KLOC // BS
    scl, r6 = stats
    x3 = xt.rearrange("p f (nb b) -> p (f nb) b", b=BS)

    # bufs=1: the WAR hazard on the single t buffer pins the scheduler to
    # strict dve1 -> dve2 pairing per chunk (it otherwise interleaves
    # chunks on the DVE queue, delaying the chunk's transpose)
    t = workt.tile([128, nf, KLOC], FP32, name=f"t{nf}", tag=f"t{nf}")
    t3 = t.rearrange("p f (nb b) -> p (f nb) b", b=BS)
    nc.vector._custom_dve(
        NVFP4_Q,
        out=t3,
        in0=x3,
        in1=r6.unsqueeze(2).broadcast_to((128, nb, BS)),
        s0=4.0,
        s1=C_VELT,
    )

    dq = work.tile([128, nf, KLOC], FP16, name=f"dq{nf}", tag=f"dq{nf}")
    dq3 = dq.rearrange("p f (nb b) -> p (f nb) b", b=BS)
    scl_b = scl.unsqueeze(2).broadcast_to((128, nb, BS))
    nc.vector._custom_dve(NVFP4_FIXDQ, out=dq3, in0=t3, in1=scl_b, s0=C_FIX32)

    # one whole-chunk xbar transpose: source tile i = in[:, 128i:128i+128]
    # lands at dst tile i, i.e. tile (f, s) at [:, base + f, s, :].
    nc.sync.dma_start_transpose(
        dst_t[:, base : base + nf, :, :].rearrange("p f s c -> p (f s) c"),
        dq.rearrange("p f k -> p (f k)"),
    )


def _body(nc, tc, x_d, w_d, o_d):
    with (
        tc.tile_pool(name="persist", bufs=1) as persist,
        tc.tile_pool(name="io", bufs=3) as io,
        tc.tile_pool(name="workt", bufs=1) as workt,
        tc.tile_pool(name="work", bufs=2) as work,
        tc.tile_pool(name="small", bufs=2) as small,
        tc.tile_pool(name="out", bufs=3) as outp,
        tc.tile_pool(name="psum", bufs=1, space="PSUM") as psum_pool,
    ):
        # tile-blocked transposed operands: tile (row-tile f, k-slice s)
        # lives at [:, f, s, :] — flat offset 512f + 128s, matching the
        # source-tile order of a whole-mega xbar transpose
        xdqT = persist.tile([128, MEGA, NSL, 128], FP16)
        wdqT = persist.tile([128, N // 128, NSL, 128], FP16)
        pools = (io, workt, work, small)

        # loads on the SWDGE queue: x halves first (they gate the first
        # matmuls), then w megas; the SWDGE descriptor rings drain in
        # issue order so earlier loads complete first
        def _load(eng, src, nf, name):
            r = io.tile([128, nf, KLOC], FP32, name=name, tag=f"raw{nf}")
            eng.dma_start(r, src.rearrange("(f p) k -> p f k", p=128))
            return r

        xh0 = _load(nc.gpsimd, x_d[0:512, :], 4, "xh0")
        w_raws = [None] * WCH
        w_raws[0] = _load(nc.gpsimd, w_d[0 : 128 * MEGA, :], MEGA, "wraw0")
        xh1 = _load(nc.gpsimd, x_d[512:1024, :], 4, "xh1")
        for j in range(1, WCH):
            w_raws[j] = _load(
                nc.gpsimd, w_d[128 * MEGA * j : 128 * MEGA * (j + 1), :], MEGA, f"wraw{j}"
            )

        st_xh0 = _stats(nc, pools, xh0, 4, "xh0")
        st_w = [None] * WCH
        st_w[0] = _stats(nc, pools, w_raws[0], MEGA, "w0")
        st_xh1 = _stats(nc, pools, xh1, 4, "xh1")

        _palette(nc, pools, xh0, st_xh0, xdqT, 0, 4)
        _palette(nc, pools, w_raws[0], st_w[0], wdqT, 0, MEGA)
        _palette(nc, pools, xh1, st_xh1, xdqT, 4, 4)

        for j in range(WCH):
            if j > 0:
                st_w[j] = _stats(nc, pools, w_raws[j], MEGA, f"w{j}")
                _palette(nc, pools, w_raws[j], st_w[j], wdqT, MEGA * j, MEGA)
            for tp in range(4 * j, 4 * (j + 1)):  # band pairs
                ps = psum_pool.tile(
                    [128, 2 * M], FP32, name=f"ps{tp % 2}", tag=f"ps{tp % 2}"
                )
                for ti in range(2):
                    t = 2 * tp + ti
                    if j == 0:
                        # mh-outer during the fill: mh=0 only needs the
                        # first x half, so matmuls start before xh1 lands
                        order = [(mh, s) for mh in range(2) for s in range(NSL)]
                    else:
                        # s-outer steady state: one LD_WEIGHTS per (t, s)
                        order = [(mh, s) for s in range(NSL) for mh in range(2)]
                    for mh, s in order:
                        nc.tensor.matmul(
                            ps[:, ti * M + mh * 512 : ti * M + (mh + 1) * 512],
                            wdqT[:, t, s, :],
                            xdqT[:, 4 * mh : 4 * (mh + 1), s, :],
                            start=(s == 0),
                            stop=(s == NSL - 1),
                        )
                ot = outp.tile([128, 2 * M], FP16, name="ot", tag="ot")
                nc.scalar.copy(ot, ps)
                nc.gpsimd.dma_start(
                    o_d[256 * tp : 256 * (tp + 1), :].rearrange(
                        "(f p) m -> p f m", p=128
                    ),
                    ot.rearrange("p (f m) -> p f m", f=2),
                )


def _get_nc():
    if "nc" not in _NC_CACHE:
        nc = bacc.Bacc(
            "TRN2", target_bir_lowering=False, debug=False, num_devices=NCORES
        )
        x_d = nc.dram_tensor("x", (M, KLOC), FP32, kind="ExternalInput").ap()
        w_d = nc.dram_tensor("w", (N, KLOC), FP32, kind="ExternalInput").ap()
        o_d = nc.dram_tensor("out", (N, M), FP16, kind="ExternalOutput").ap()
        with tile.TileContext(nc) as tc:
            _body(nc, tc, x_d, w_d, o_d)
        nc.compile()
        _NC_CACHE["nc"] = nc
    return _NC_CACHE["nc"]


def kernel(x: np.ndarray, weight: np.ndarray, _trace: bool = False, **_):
    nc = _get_nc()
    x = np.ascontiguousarray(x, dtype=np.float32)
    weight = np.ascontiguousarray(weight, dtype=np.float32)
    in_maps = [
        {
            "x": x[:, c * KLOC : (c + 1) * KLOC],
            "w": weight[:, c * KLOC : (c + 1) * KLOC],
        }
        for c in range(NCORES)
    ]
    res = bass_utils.run_bass_kernel_spmd(
        nc, in_maps, list(range(NCORES)), trace=_trace
    )
    acc = np.zeros((N, M), dtype=np.float32)
    for c in range(NCORES):
        acc += res.results[c]["out"].astype(np.float32)
    if _trace:
        kernel.last_result = res
    return np.ascontiguousarray(acc.T, dtype=np.float32)


# revision 28
# speedup vs baseline: 1.0812x; 1.0704x over previous
"""NVFP4 quantized linear (simulated) on 8 TRN2 NeuronCores.

out = dq(quant_nvfp4(x)) @ dq(quant_nvfp4(w)).T

Sharding: K-parallel (contraction dim). Core c gets x[:, 512c:512c+512]
and weight[:, 512c:512c+512]. NVFP4 blocks are 32 wide along K, so
quantization is fully local to a K-slice for BOTH operands; there are no
collectives at all (no AllGather barrier, no launch-skew sensitivity).
Each core computes the full-size partial product over its K-slice,
drains it transposed ([N, M] fp16), and the host sums the 8 partials
(the unshard step) and transposes back.

Quant runs on [128, 8, 512] megachunks (one DMA load each; 1 for x, 4
for w). Palette rounding (NVFP4_Q custom DVE op, 8 stages): v = x*r6;
hi = Veltkamp 2-sig-bit round; t = select(v^2<=4, v, hi). The remaining
snap-to-0.5-grid + dequant scale is routed per-megachunk to balance
engines:
  - DVE route: one 3-stage custom op
        dq16 = ((t + 1.5*2^22) - 1.5*2^22) * scl_bcast
    (the fp32 magic add rounds t to the 0.5 grid; DVE stages round to
    fp32 like the Veltkamp stages do).
  - Scalar+Pool route: y16 = fp16(t + 768) — 1.5*2^9 makes the fp16
    downconvert round to the 0.5 grid — then z16 = y16 - 768 (exact) on
    Scalar, and dq = z16 * scl_bcast on GpSimd.

The transposed operands use a tile-blocked layout (tile (f, s) of a
megachunk lands at flat offset 128*(4f+s)) so one xbar-transpose DMA
moves a whole megachunk — 5 transpose triggers total instead of 40
(HWDGE trigger cost ~1.2us each on the sync queue was serializing the
pipeline). Matmuls read the blocked layout via strided APs; PSUM holds
two bands per tile (4 banks) so drains/stores batch pairwise.
"""

import dataclasses
import sys

import numpy as np

if "/opt/trn_rl_repo" not in sys.path:
    sys.path.insert(0, "/opt/trn_rl_repo")

from concourse import bacc, mybir
from concourse import dve_ops as _dve_ops
import concourse.bass as bass  # noqa: F401
import concourse.tile as tile
import concourse.bass_utils as bass_utils
from concourse.dve_spec import Spec, Src0, Src1, C0, C1, select, sq, lower
from concourse.dve_uop import DveOpSpec

M, K, N = 1024, 4096, 4096
NCORES = 8
KLOC = K // NCORES  # 512 contraction elements per core
BS = 32
NSL = KLOC // 128  # 4 transposed k-slices per core
MEGA = 8  # row-tiles per megachunk
NBM = MEGA * KLOC // BS  # 128 blocks per megachunk
WCH = N // (128 * MEGA)  # 4 w megachunks

FP32 = mybir.dt.float32
FP16 = mybir.dt.float16
Alu = mybir.AluOpType
AX = mybir.AxisListType

C_VELT = 4194305.0  # 2^22 + 1: Veltkamp split -> 2 significant bits
C_FIX32 = 6291456.0  # 1.5 * 2^22: fp32 magic add rounds to the 0.5-grid
C_FIX16 = 768.0  # 1.5 * 2^9: fp16 magic; downconvert snaps to 0.5-grid

_NC_CACHE = {}


def _nvfp4_ref(in0, in1, c0, c1, c2):
    f32 = np.float32
    x = np.asarray(in0, np.float32)
    r6 = np.asarray(in1, np.float32)
    if r6.shape != x.shape:
        if r6.ndim == 3:
            r6 = r6[..., 0]
        bs = x.size // r6.size
        r6 = np.repeat(r6, bs, axis=-1).reshape(x.shape)
    v = (x * r6).astype(np.float32)
    c = (v * f32(c1)).astype(np.float32)
    d = (c - v).astype(np.float32)
    hi = (c - d).astype(np.float32)
    return np.where(v * v <= np.asarray(c0, np.float32), v, hi).astype(np.float32)


def _fixdq_ref(in0, in1, c0, c1, c2):
    t = np.asarray(in0, np.float32)
    s = np.asarray(in1, np.float32)
    if s.shape != t.shape:
        if s.ndim == 3:
            s = s[..., 0]
        bs = t.size // s.size
        s = np.repeat(s, bs, axis=-1).reshape(t.shape)
    c = np.float32(c0)
    a = (t + c).astype(np.float32)
    b = (a - c).astype(np.float32)
    return (b * s).astype(np.float32)


def _register_op(name, spec):
    if name in _dve_ops._SUB_OPCODE_FOR_NAME:
        return next(o for o in _dve_ops.OPS if o.name == name)
    op = _dve_ops.DveOp(name, spec, subdim=False, uops_sha={})
    _dve_ops.OPS.append(op)
    _dve_ops.CUSTOM_DVE_SPECS[name] = spec
    row = _dve_ops._CUSTOM_DVE_ROW_BASE + len(_dve_ops.OPS) - 1
    _dve_ops._SUB_OPCODE_FOR_NAME[name] = row
    shas = {}
    for ver in ("v3",):
        s = DveOpSpec(name=name, opcode=row, uops=lower(spec, ver=ver), rd1_en=True)
        shas[ver] = s.sha(ver)
    op = dataclasses.replace(op, uops_sha=shas)
    _dve_ops.OPS[-1] = op
    _dve_ops.CUSTOM_DVE_SPECS[name] = op.spec
    return op


def _make_ops():
    _v = Src0 * Src1
    _c = _v * C1
    _d = _c - _v
    _hi = _c - _d
    _m = sq(_v) <= C0
    q = _register_op("NVFP4_Q_ANT", Spec(body=select(_m, _v, _hi), reference=_nvfp4_ref))
    _a = Src0 + C0
    _b = _a - C0
    f = _register_op("NVFP4_FIXDQ_ANT", Spec(body=_b * Src1, reference=_fixdq_ref))
    return q, f


NVFP4_Q, NVFP4_FIXDQ = _make_ops()


def _stats(nc, pools, xt, nf, uid):
    """Blockwise stats for a pre-loaded [128, nf, KLOC] fp32 tile:
    returns (scl, r6). Cheap; issued early so they hide in load gaps and
    keep the DVE-counter positions of the palette passes exact."""
    io, workt, work, small = pools
    nb = nf * KLOC // BS
    x4 = xt.rearrange("p f (nb b) -> p f nb b", b=BS)
    bmax = small.tile([128, nb], FP32, name=f"bmax{uid}", tag=f"bmax{nf}")
    nc.vector.tensor_reduce(
        bmax.rearrange("p (f nb) -> p f nb", f=nf),
        x4,
        axis=AX.X,
        op=Alu.max,
        apply_absolute_value=True,
    )
    scl = small.tile([128, nb], FP32, name=f"scl{uid}", tag=f"scl{uid}")
    nc.vector.tensor_scalar(scl, bmax, 1e-12, 1.0 / 6.0, Alu.max, Alu.mult)
    r6 = small.tile([128, nb], FP32, name=f"r6{uid}", tag=f"r6{uid}")
    nc.vector.reciprocal_approx_fast(r6, scl)
    return scl, r6


def _palette(nc, pools, xt, stats, dst_t, base, nf):
    """Palette-round + fix + dequant a [128, nf, KLOC] tile on the DVE and
    xbar-transpose it into the tile-blocked dst_t[:, base : base+nf]."""
    io, workt, work, small = pools
    nb = nf * KLOC // BS
    scl, r6 = stats
    x3 = xt.rearrange("p f (nb b) -> p (f nb) b", b=BS)

    # bufs=1: the WAR hazard on the single t buffer pins the scheduler to
    # strict dve1 -> dve2 pairing per chunk (it otherwise interleaves
    # chunks on the DVE queue, delaying the chunk's transpose)
    t = workt.tile([128, nf, KLOC], FP32, name=f"t{nf}", tag=f"t{nf}")
    t3 = t.rearrange("p f (nb b) -> p (f nb) b", b=BS)
    nc.vector._custom_dve(
        NVFP4_Q,
        out=t3,
        in0=x3,
        in1=r6.unsqueeze(2).broadcast_to((128, nb, BS)),
        s0=4.0,
        s1=C_VELT,
    )

    dq = work.tile([128, nf, KLOC], FP16, name=f"dq{nf}", tag=f"dq{nf}")
    dq3 = dq.rearrange("p f (nb b) -> p (f nb) b", b=BS)
    scl_b = scl.unsqueeze(2).broadcast_to((128, nb, BS))
    nc.vector._custom_dve(NVFP4_FIXDQ, out=dq3, in0=t3, in1=scl_b, s0=C_FIX32)

    # one whole-chunk xbar transpose: source tile i = in[:, 128i:128i+128]
    # lands at dst tile i, i.e. tile (f, s) at [:, base + f, s, :].
    nc.sync.dma_start_transpose(
        dst_t[:, base : base + nf, :, :].rearrange("p f s c -> p (f s) c"),
        dq.rearrange("p f k -> p (f k)"),
    )


def _body(nc, tc, x_d, w_d, o_d):
    with (
        tc.tile_pool(name="persist", bufs=1) as persist,
        tc.tile_pool(name="io", bufs=1) as io,
        tc.tile_pool(name="workt", bufs=1) as workt,
        tc.tile_pool(name="work", bufs=3) as work,
        tc.tile_pool(name="small", bufs=1) as small,
        tc.tile_pool(name="out", bufs=2) as outp,
        tc.tile_pool(name="psum", bufs=1, space="PSUM") as psum_pool,
    ):
        # tile-blocked transposed operands: tile (row-tile f, k-slice s)
        # lives at [:, f, s, :] — flat offset 512f + 128s, matching the
        # source-tile order of a whole-mega xbar transpose
        xdqT = persist.tile([128, MEGA, NSL, 128], FP16)
        wdqT = persist.tile([128, N // 128, NSL, 128], FP16)
        pools = (io, workt, work, small)

        # loads on the SWDGE queue: x halves first (they gate the first
        # matmuls), then w megas; the SWDGE descriptor rings drain in
        # issue order so earlier loads complete first
        def _load(eng, src, nf, name):
            r = io.tile([128, nf, KLOC], FP32, name=name, tag=name)
            eng.dma_start(r, src.rearrange("(f p) k -> p f k", p=128))
            return r

        # every load gets its own buffer so none waits on a WAR — an
        # in-flight late load stalls unrelated DMAs that share its hw queue
        w_raws = [None] * WCH
        w_raws[0] = _load(nc.gpsimd, w_d[0 : 128 * MEGA, :], MEGA, "wraw0")
        xh0 = _load(nc.gpsimd, x_d[0:512, :], 4, "xh0")
        xh1 = _load(nc.gpsimd, x_d[512:1024, :], 4, "xh1")
        for j in range(1, WCH):
            w_raws[j] = _load(
                nc.gpsimd, w_d[128 * MEGA * j : 128 * MEGA * (j + 1), :], MEGA, f"wraw{j}"
            )

        # w0's chain first: it gates the first matmul
        st_w = [None] * WCH
        st_w[0] = _stats(nc, pools, w_raws[0], MEGA, "w0")
        st_xh0 = _stats(nc, pools, xh0, 4, "xh0")
        st_xh1 = _stats(nc, pools, xh1, 4, "xh1")

        _palette(nc, pools, w_raws[0], st_w[0], wdqT, 0, MEGA)
        _palette(nc, pools, xh0, st_xh0, xdqT, 0, 4)
        _palette(nc, pools, xh1, st_xh1, xdqT, 4, 4)

        for j in range(1, WCH):
            st_w[j] = _stats(nc, pools, w_raws[j], MEGA, f"w{j}")

        for j in range(WCH):
            if j > 0:
                _palette(nc, pools, w_raws[j], st_w[j], wdqT, MEGA * j, MEGA)
            for tp in range(4 * j, 4 * (j + 1)):  # band pairs
                ps = psum_pool.tile(
                    [128, 2 * M], FP32, name=f"ps{tp % 2}", tag=f"ps{tp % 2}"
                )
                for ti in range(2):
                    t = 2 * tp + ti
                    if j == 0:
                        # mh-outer during the fill: mh=0 only needs the
                        # first x half, so matmuls start before xh1 lands
                        order = [(mh, s) for mh in range(2) for s in range(NSL)]
                    else:
                        # s-outer steady state: one LD_WEIGHTS per (t, s)
                        order = [(mh, s) for s in range(NSL) for mh in range(2)]
                    for mh, s in order:
                        nc.tensor.matmul(
                            ps[:, ti * M + mh * 512 : ti * M + (mh + 1) * 512],
                            wdqT[:, t, s, :],
                            xdqT[:, 4 * mh : 4 * (mh + 1), s, :],
                            start=(s == 0),
                            stop=(s == NSL - 1),
                        )
                if tp % 2 == 0:
                    ot = outp.tile([128, 4 * M], FP16, name="ot", tag="ot")
                nc.scalar.copy(ot[:, (tp % 2) * 2 * M : (tp % 2 + 1) * 2 * M], ps)
                if tp % 2 == 1:
                    # one store per 4 bands (2 drained pairs)
                    nc.gpsimd.dma_start(
                        o_d[256 * (tp - 1) : 256 * (tp + 1), :].rearrange(
                            "(f p) m -> p f m", p=128
                        ),
                        ot.rearrange("p (f m) -> p f m", f=4),
                    )


def _get_nc():
    if "nc" not in _NC_CACHE:
        nc = bacc.Bacc(
            "TRN2", target_bir_lowering=False, debug=False, num_devices=NCORES
        )
        x_d = nc.dram_tensor("x", (M, KLOC), FP32, kind="ExternalInput").ap()
        w_d = nc.dram_tensor("w", (N, KLOC), FP32, kind="ExternalInput").ap()
        o_d = nc.dram_tensor("out", (N, M), FP16, kind="ExternalOutput").ap()
        with tile.TileContext(nc) as tc:
            _body(nc, tc, x_d, w_d, o_d)
        nc.compile()
        _NC_CACHE["nc"] = nc
    return _NC_CACHE["nc"]


def kernel(x: np.ndarray, weight: np.ndarray, _trace: bool = False, **_):
    nc = _get_nc()
    x = np.ascontiguousarray(x, dtype=np.float32)
    weight = np.ascontiguousarray(weight, dtype=np.float32)
    in_maps = [
        {
            "x": x[:, c * KLOC : (c + 1) * KLOC],
            "w": weight[:, c * KLOC : (c + 1) * KLOC],
        }
        for c in range(NCORES)
    ]
    res = bass_utils.run_bass_kernel_spmd(
        nc, in_maps, list(range(NCORES)), trace=_trace
    )
    acc = np.zeros((N, M), dtype=np.float32)
    for c in range(NCORES):
        acc += res.results[c]["out"].astype(np.float32)
    if _trace:
        kernel.last_result = res
    return np.ascontiguousarray(acc.T, dtype=np.float32)


# revision 31
# speedup vs baseline: 1.1021x; 1.0193x over previous
"""NVFP4 quantized linear (simulated) on 8 TRN2 NeuronCores.

out = dq(quant_nvfp4(x)) @ dq(quant_nvfp4(w)).T

Sharding: K-parallel (contraction dim). Core c gets x[:, 512c:512c+512]
and weight[:, 512c:512c+512]. NVFP4 blocks are 32 wide along K, so
quantization is fully local to a K-slice for BOTH operands; there are no
collectives at all (no AllGather barrier, no launch-skew sensitivity).
Each core computes the full-size partial product over its K-slice,
drains it transposed ([N, M] fp16), and the host sums the 8 partials
(the unshard step) and transposes back.

Quant runs on [128, 8, 512] megachunks (one DMA load each; 1 for x, 4
for w). Palette rounding (NVFP4_Q custom DVE op, 8 stages): v = x*r6;
hi = Veltkamp 2-sig-bit round; t = select(v^2<=4, v, hi). The remaining
snap-to-0.5-grid + dequant scale is routed per-megachunk to balance
engines:
  - DVE route: one 3-stage custom op
        dq16 = ((t + 1.5*2^22) - 1.5*2^22) * scl_bcast
    (the fp32 magic add rounds t to the 0.5 grid; DVE stages round to
    fp32 like the Veltkamp stages do).
  - Scalar+Pool route: y16 = fp16(t + 768) — 1.5*2^9 makes the fp16
    downconvert round to the 0.5 grid — then z16 = y16 - 768 (exact) on
    Scalar, and dq = z16 * scl_bcast on GpSimd.

The transposed operands use a tile-blocked layout (tile (f, s) of a
megachunk lands at flat offset 128*(4f+s)) so one xbar-transpose DMA
moves a whole megachunk — 5 transpose triggers total instead of 40
(HWDGE trigger cost ~1.2us each on the sync queue was serializing the
pipeline). Matmuls read the blocked layout via strided APs; PSUM holds
two bands per tile (4 banks) so drains/stores batch pairwise.
"""

import dataclasses
import sys

import numpy as np

if "/opt/trn_rl_repo" not in sys.path:
    sys.path.insert(0, "/opt/trn_rl_repo")

from concourse import bacc, mybir
from concourse import dve_ops as _dve_ops
import concourse.bass as bass  # noqa: F401
import concourse.tile as tile
import concourse.bass_utils as bass_utils
from concourse.dve_spec import Spec, Src0, Src1, C0, C1, select, sq, lower
from concourse.dve_uop import DveOpSpec

M, K, N = 1024, 4096, 4096
NCORES = 8
KLOC = K // NCORES  # 512 contraction elements per core
BS = 32
NSL = KLOC // 128  # 4 transposed k-slices per core
MEGA = 8  # row-tiles per megachunk
NBM = MEGA * KLOC // BS  # 128 blocks per megachunk
WCH = N // (128 * MEGA)  # 4 w megachunks

FP32 = mybir.dt.float32
FP16 = mybir.dt.float16
Alu = mybir.AluOpType
AX = mybir.AxisListType

C_VELT = 4194305.0  # 2^22 + 1: Veltkamp split -> 2 significant bits
C_FIX32 = 6291456.0  # 1.5 * 2^22: fp32 magic add rounds to the 0.5-grid
C_FIX16 = 768.0  # 1.5 * 2^9: fp16 magic; downconvert snaps to 0.5-grid

_NC_CACHE = {}


def _nvfp4_ref(in0, in1, c0, c1, c2):
    f32 = np.float32
    x = np.asarray(in0, np.float32)
    r6 = np.asarray(in1, np.float32)
    if r6.shape != x.shape:
        if r6.ndim == 3:
            r6 = r6[..., 0]
        bs = x.size // r6.size
        r6 = np.repeat(r6, bs, axis=-1).reshape(x.shape)
    v = (x * r6).astype(np.float32)
    c = (v * f32(c1)).astype(np.float32)
    d = (c - v).astype(np.float32)
    hi = (c - d).astype(np.float32)
    return np.where(v * v <= np.asarray(c0, np.float32), v, hi).astype(np.float32)


def _fixdq_ref(in0, in1, c0, c1, c2):
    t = np.asarray(in0, np.float32)
    s = np.asarray(in1, np.float32)
    if s.shape != t.shape:
        if s.ndim == 3:
            s = s[..., 0]
        bs = t.size // s.size
        s = np.repeat(s, bs, axis=-1).reshape(t.shape)
    c = np.float32(c0)
    a = (t + c).astype(np.float32)
    b = (a - c).astype(np.float32)
    return (b * s).astype(np.float32)


def _register_op(name, spec):
    if name in _dve_ops._SUB_OPCODE_FOR_NAME:
        return next(o for o in _dve_ops.OPS if o.name == name)
    op = _dve_ops.DveOp(name, spec, subdim=False, uops_sha={})
    _dve_ops.OPS.append(op)
    _dve_ops.CUSTOM_DVE_SPECS[name] = spec
    row = _dve_ops._CUSTOM_DVE_ROW_BASE + len(_dve_ops.OPS) - 1
    _dve_ops._SUB_OPCODE_FOR_NAME[name] = row
    shas = {}
    for ver in ("v3",):
        s = DveOpSpec(name=name, opcode=row, uops=lower(spec, ver=ver), rd1_en=True)
        shas[ver] = s.sha(ver)
    op = dataclasses.replace(op, uops_sha=shas)
    _dve_ops.OPS[-1] = op
    _dve_ops.CUSTOM_DVE_SPECS[name] = op.spec
    return op


def _make_ops():
    _v = Src0 * Src1
    _c = _v * C1
    _d = _c - _v
    _hi = _c - _d
    _m = sq(_v) <= C0
    q = _register_op("NVFP4_Q_ANT", Spec(body=select(_m, _v, _hi), reference=_nvfp4_ref))
    _a = Src0 + C0
    _b = _a - C0
    f = _register_op("NVFP4_FIXDQ_ANT", Spec(body=_b * Src1, reference=_fixdq_ref))
    return q, f


NVFP4_Q, NVFP4_FIXDQ = _make_ops()


def _stats(nc, pools, xt, nf, uid):
    """Blockwise stats for a pre-loaded [128, nf, KLOC] fp32 tile:
    returns (scl, r6). Cheap; issued early so they hide in load gaps and
    keep the DVE-counter positions of the palette passes exact."""
    io, workt, work, small = pools
    nb = nf * KLOC // BS
    x4 = xt.rearrange("p f (nb b) -> p f nb b", b=BS)
    bmax = small.tile([128, nb], FP32, name=f"bmax{uid}", tag=f"bmax{nf}")
    nc.vector.tensor_reduce(
        bmax.rearrange("p (f nb) -> p f nb", f=nf),
        x4,
        axis=AX.X,
        op=Alu.max,
        apply_absolute_value=True,
    )
    scl = small.tile([128, nb], FP32, name=f"scl{uid}", tag=f"scl{uid}")
    nc.vector.tensor_scalar(scl, bmax, 1e-12, 1.0 / 6.0, Alu.max, Alu.mult)
    r6 = small.tile([128, nb], FP32, name=f"r6{uid}", tag=f"r6{uid}")
    nc.vector.reciprocal_approx_fast(r6, scl)
    return scl, r6


def _palette(nc, pools, xt, stats, dst_t, base, nf, split=False):
    """Palette-round + fix + dequant a [128, nf, KLOC] tile on the DVE and
    xbar-transpose it into the tile-blocked dst_t[:, base : base+nf].
    split=True transposes in two halves so the first bands land sooner."""
    io, workt, work, small = pools
    nb = nf * KLOC // BS
    scl, r6 = stats
    x3 = xt.rearrange("p f (nb b) -> p (f nb) b", b=BS)

    # bufs=1: the WAR hazard on the single t buffer pins the scheduler to
    # strict dve1 -> dve2 pairing per chunk (it otherwise interleaves
    # chunks on the DVE queue, delaying the chunk's transpose)
    t = workt.tile([128, nf, KLOC], FP32, name=f"t{nf}", tag=f"t{nf}")
    t3 = t.rearrange("p f (nb b) -> p (f nb) b", b=BS)
    nc.vector._custom_dve(
        NVFP4_Q,
        out=t3,
        in0=x3,
        in1=r6.unsqueeze(2).broadcast_to((128, nb, BS)),
        s0=4.0,
        s1=C_VELT,
    )

    dq = work.tile([128, nf, KLOC], FP16, name=f"dq{nf}", tag=f"dq{nf}")
    dq3 = dq.rearrange("p f (nb b) -> p (f nb) b", b=BS)
    scl_b = scl.unsqueeze(2).broadcast_to((128, nb, BS))
    nc.vector._custom_dve(NVFP4_FIXDQ, out=dq3, in0=t3, in1=scl_b, s0=C_FIX32)

    # one whole-chunk xbar transpose: source tile i = in[:, 128i:128i+128]
    # lands at dst tile i, i.e. tile (f, s) at [:, base + f, s, :].
    halves = 2 if split else 1
    hf = nf // halves
    for h in range(halves):
        nc.sync.dma_start_transpose(
            dst_t[:, base + h * hf : base + (h + 1) * hf, :, :].rearrange(
                "p f s c -> p (f s) c"
            ),
            dq[:, h * hf : (h + 1) * hf, :].rearrange("p f k -> p (f k)"),
        )


def _body(nc, tc, x_d, w_d, o_d):
    with (
        tc.tile_pool(name="persist", bufs=1) as persist,
        tc.tile_pool(name="io", bufs=1) as io,
        tc.tile_pool(name="workt", bufs=1) as workt,
        tc.tile_pool(name="work", bufs=3) as work,
        tc.tile_pool(name="small", bufs=1) as small,
        tc.tile_pool(name="out", bufs=2) as outp,
        tc.tile_pool(name="psum", bufs=1, space="PSUM") as psum_pool,
    ):
        # tile-blocked transposed operands: tile (row-tile f, k-slice s)
        # lives at [:, f, s, :] — flat offset 512f + 128s, matching the
        # source-tile order of a whole-mega xbar transpose
        xdqT = persist.tile([128, MEGA, NSL, 128], FP16)
        wdqT = persist.tile([128, N // 128, NSL, 128], FP16)
        pools = (io, workt, work, small)

        # loads on the SWDGE queue: x halves first (they gate the first
        # matmuls), then w megas; the SWDGE descriptor rings drain in
        # issue order so earlier loads complete first
        def _load(eng, src, nf, name):
            r = io.tile([128, nf, KLOC], FP32, name=name, tag=name)
            eng.dma_start(r, src.rearrange("(f p) k -> p f k", p=128))
            return r

        # every load gets its own buffer so none waits on a WAR — an
        # in-flight late load stalls unrelated DMAs that share its hw queue
        w_raws = [None] * WCH
        w_raws[0] = _load(nc.gpsimd, w_d[0 : 128 * MEGA, :], MEGA, "wraw0")
        xh0 = _load(nc.gpsimd, x_d[0:512, :], 4, "xh0")
        xh1 = _load(nc.gpsimd, x_d[512:1024, :], 4, "xh1")
        for j in range(1, WCH):
            w_raws[j] = _load(
                nc.gpsimd, w_d[128 * MEGA * j : 128 * MEGA * (j + 1), :], MEGA, f"wraw{j}"
            )

        # w0's chain first: it gates the first matmul. ALL quant (and so
        # all transpose DMAs) is issued before any matmul/store so the
        # transposes share rotating DMA hw queues only with the early
        # loads, never with an output store (a shared queue makes the
        # transpose wait for the store's drain chain).
        st_w = [None] * WCH
        st_w[0] = _stats(nc, pools, w_raws[0], MEGA, "w0")
        st_xh0 = _stats(nc, pools, xh0, 4, "xh0")
        st_xh1 = _stats(nc, pools, xh1, 4, "xh1")

        _palette(nc, pools, w_raws[0], st_w[0], wdqT, 0, MEGA, split=True)
        _palette(nc, pools, xh0, st_xh0, xdqT, 0, 4)
        _palette(nc, pools, xh1, st_xh1, xdqT, 4, 4)

        for j in range(1, WCH):
            st_w[j] = _stats(nc, pools, w_raws[j], MEGA, f"w{j}")
            _palette(nc, pools, w_raws[j], st_w[j], wdqT, MEGA * j, MEGA)

        for t in range(N // 128):  # 32 output bands
            ps = psum_pool.tile([128, M], FP32, name=f"ps{t % 4}", tag=f"ps{t % 4}")
            if t < 8:
                # mh-outer during the fill: mh=0 only needs the first
                # x half, so matmuls start before xh1 lands
                order = [(mh, s) for mh in range(2) for s in range(NSL)]
            else:
                # s-outer steady state: one LD_WEIGHTS per (t, s)
                order = [(mh, s) for s in range(NSL) for mh in range(2)]
            for mh, s in order:
                nc.tensor.matmul(
                    ps[:, mh * 512 : (mh + 1) * 512],
                    wdqT[:, t, s, :],
                    xdqT[:, 4 * mh : 4 * (mh + 1), s, :],
                    start=(s == 0),
                    stop=(s == NSL - 1),
                )
            if t % 4 == 0:
                ot = outp.tile([128, 4 * M], FP16, name="ot", tag="ot")
            nc.scalar.copy(ot[:, (t % 4) * M : (t % 4 + 1) * M], ps)
            if t % 4 == 3:
                # one store per 4 drained bands
                nc.gpsimd.dma_start(
                    o_d[128 * (t - 3) : 128 * (t + 1), :].rearrange(
                        "(f p) m -> p f m", p=128
                    ),
                    ot.rearrange("p (f m) -> p f m", f=4),
                )


def _get_nc():
    if "nc" not in _NC_CACHE:
        nc = bacc.Bacc(
            "TRN2", target_bir_lowering=False, debug=False, num_devices=NCORES
        )
        x_d = nc.dram_tensor("x", (M, KLOC), FP32, kind="ExternalInput").ap()
        w_d = nc.dram_tensor("w", (N, KLOC), FP32, kind="ExternalInput").ap()
        o_d = nc.dram_tensor("out", (N, M), FP16, kind="ExternalOutput").ap()
        with tile.TileContext(nc) as tc:
            _body(nc, tc, x_d, w_d, o_d)
        nc.compile()
        _NC_CACHE["nc"] = nc
    return _NC_CACHE["nc"]


def kernel(x: np.ndarray, weight: np.ndarray, _trace: bool = False, **_):
    nc = _get_nc()
    x = np.ascontiguousarray(x, dtype=np.float32)
    weight = np.ascontiguousarray(weight, dtype=np.float32)
    in_maps = [
        {
            "x": x[:, c * KLOC : (c + 1) * KLOC],
            "w": weight[:, c * KLOC : (c + 1) * KLOC],
        }
        for c in range(NCORES)
    ]
    res = bass_utils.run_bass_kernel_spmd(
        nc, in_maps, list(range(NCORES)), trace=_trace
    )
    acc = np.zeros((N, M), dtype=np.float32)
    for c in range(NCORES):
        acc += res.results[c]["out"].astype(np.float32)
    if _trace:
        kernel.last_result = res
    return np.ascontiguousarray(acc.T, dtype=np.float32)
